# revision 1
# baseline (speedup 1.0000x reference)
"""Trainium2 Bass kernel for nn_ExperimentalLayer9 (dense transformer layer).

Layer: x + gelu(attn(x) ) @ Wf with
  Q = split_heads(x), K = split_heads(x@Wk+bk), V = split_heads(x@Wv+bv)
  causal softmax (no 1/sqrt(d) scale), exact-erf gelu, residual add.

Sharding over 8 NeuronCores: 2 batch groups x 4-way head/tensor parallel.
Core c handles batch b=c//4 and heads [4r, 4r+4) with r=c%4.  Each core
computes K^T/V projections for its head slice, causal flash-style
attention in transposed-score layout, gelu, and a partial FF over its
1024-row slice of Wf.  A 4-rank ReduceScatter (bf16) sums the FF
partials within each batch group; each core adds the residual x rows for
its rank's 512-row shard and returns that shard.  The host reassembles
the [2, 2048, 1024] output.

All matmuls run in bf16 (fp32 PSUM accumulation); softmax/normalization
in fp32.  exp is computed without max-subtraction (scores are bounded:
std ~5, so exp stays well inside fp32/bf16 range) which avoids any
partition-axis max reduction.  The exp-sum l(q) is obtained for free by
appending a ones-column to V in the attention@V matmul; 1/l is then a
per-partition scalar multiply fused on the vector engine.
"""

import numpy as np
import ml_dtypes

import concourse.bass as bass
import concourse.mybir as mybir
import concourse.tile as tile
from concourse import bacc
from concourse import bass_utils

# Problem shapes (hardcoded per contest contract).
B, S, D, H, DHID = 2, 2048, 1024, 16, 4096
NCORES = 8
GROUP = 4              # cores per batch group
HPC = 4                # heads per core
DK = 64                # q/k head dim
DV = 256               # v head dim
DKS = HPC * DK         # 256  k-slice per core
DVS = HPC * DV         # 1024 v/hidden slice per core
ROWS = S // GROUP      # 512  output rows per core after ReduceScatter
NM = D // 128          # 8    contraction chunks over d_model
VSTRIDE = DV + 1       # 257  V columns per head incl. ones column

BF16 = mybir.dt.bfloat16
F32 = mybir.dt.float32
AF = mybir.ActivationFunctionType

bf16 = ml_dtypes.bfloat16

_compiled = None


def build_program():
    nc = bacc.Bacc(
        "TRN2",
        target_bir_lowering=False,
        debug=False,
        enable_asserts=True,
        num_devices=NCORES,
    )

    # Per-core inputs (values differ per core; program is SPMD-identical).
    xT = nc.dram_tensor("xT", [D, S], BF16, kind="ExternalInput").ap()
    qT = nc.dram_tensor("qT", [DKS, S], BF16, kind="ExternalInput").ap()
    xres = nc.dram_tensor("xres", [ROWS, D], F32, kind="ExternalInput").ap()
    wk = nc.dram_tensor("wk", [D, DKS], BF16, kind="ExternalInput").ap()
    wv = nc.dram_tensor("wv", [D, DVS], BF16, kind="ExternalInput").ap()
    wf = nc.dram_tensor("wf", [DVS, D], BF16, kind="ExternalInput").ap()
    bkb = nc.dram_tensor("bkb", [1, DKS], BF16, kind="ExternalInput").ap()
    bvb = nc.dram_tensor("bvb", [1, DVS], BF16, kind="ExternalInput").ap()
    maskt = nc.dram_tensor("maskt", [128, 128], BF16, kind="ExternalInput").ap()
    ident = nc.dram_tensor("ident", [128, 128], BF16, kind="ExternalInput").ap()
    onesr = nc.dram_tensor("onesr", [1, 512], BF16, kind="ExternalInput").ap()
    out = nc.dram_tensor("out", [ROWS, D], F32, kind="ExternalOutput").ap()

    with tile.TileContext(nc) as tc:
        _body(nc, tc, xT, qT, xres, wk, wv, wf, bkb, bvb, maskt, ident, onesr, out)

    nc.compile()
    return nc


def _body(nc, tc, xT, qT, xres, wk, wv, wf, bkb, bvb, maskt, ident, onesr, out):
    NST = S // 128     # 16 s tiles of 128
    NQT2 = S // 1024   # 2  q tiles of 1024

    with (
        tc.tile_pool(name="const", bufs=1) as constp,
        tc.tile_pool(name="kv", bufs=1) as kvp,
        tc.tile_pool(name="got", bufs=1) as gotp,
        tc.tile_pool(name="res", bufs=1) as resp,
        tc.tile_pool(name="rfp", bufs=2) as rfp,
        tc.tile_pool(name="small", bufs=8) as smallp,
        tc.tile_pool(name="dram", bufs=1, space="DRAM") as dramp,
    ):
        # ---- constants (ACT queue) ------------------------------------
        ones_sb = constp.tile([1, 512], BF16)
        nc.scalar.dma_start(ones_sb[:], onesr[:])
        mask_sb = constp.tile([128, 128], BF16)
        nc.scalar.dma_start(mask_sb[:], maskt[:])
        bk_sb = constp.tile([1, DKS], BF16)
        nc.scalar.dma_start(bk_sb[:], bkb[:])
        bv_sb = constp.tile([1, DVS], BF16)
        nc.scalar.dma_start(bv_sb[:], bvb[:])

        # Warm up the collectives path (ncfw/channel setup) so the first
        # real ReduceScatter doesn't pay ~25us of first-call overhead.
        warm_in = dramp.tile([4, 16], BF16, tag="warm_in")
        warm_out = dramp.tile([1, 16], BF16, tag="warm_out")
        nc.scalar.dma_start(
            warm_in[:].rearrange("a b -> (a b)")[None, :], ones_sb[0:1, 0:64]
        )
        nc.gpsimd.collective_compute(
            "ReduceScatter",
            mybir.AluOpType.add,
            replica_groups=[[0, 1, 2, 3], [4, 5, 6, 7]],
            ins=[warm_in.opt()],
            outs=[warm_out.opt()],
        )

        # [1024, n] DRAM -> [128, 8*n] SBUF, per-chunk DMAs on the Sync
        # queue (all complete before the first xbar transpose issues)
        def load_chunked(pool, src, n):
            t = pool.tile([128, NM * n], src.dtype)
            for m in range(NM):
                nc.sync.dma_start(
                    t[:, m * n : (m + 1) * n],
                    src[m * 128 : (m + 1) * 128, :],
                )
            return t

        qT_sb = kvp.tile([128, 2 * S], BF16)
        for m in range(2):
            nc.sync.dma_start(
                qT_sb[:, m * S : (m + 1) * S], qT[m * 128 : (m + 1) * 128, :]
            )
        kt_sb = kvp.tile([128, 2 * S], BF16)   # K^T rows dk%128, chunk dk//128
        v_sb = kvp.tile([128, NST * HPC * VSTRIDE], BF16)
        got_sb = gotp.tile([128, NM * S], BF16)  # gelu(o)^T, hc-major x q
        # residual x rows: no deps, load early (ACT queue)
        xrs = []
        for g in range(4):
            xr = resp.tile([128, D], F32, tag=f"xr{g}")
            nc.scalar.dma_start(xr[:], xres[g * 128 : (g + 1) * 128, :])
            xrs.append(xr)

        # ---- projections ---------------------------------------------
        with (
            tc.tile_pool(name="projw", bufs=1) as pwp,
            tc.tile_pool(name="xt", bufs=1) as xtp,
            tc.tile_pool(name="psProj", bufs=4, space="PSUM") as psP,
        ):
            wk_sb = load_chunked(pwp, wk, DKS)
            xT_sb = load_chunked(xtp, xT, S)
            wv_sb = load_chunked(pwp, wv, DVS)

            # K^T[dk, s]: lhsT = Wk chunk [128m, 128dk], rhs = xT chunk [128m, 512s]
            for dkt in range(2):
                for st in range(4):
                    ps = psP.tile([128, 512], F32, tag="proj")
                    nc.tensor.matmul(
                        ps[:],
                        bk_sb[:, dkt * 128 : (dkt + 1) * 128],
                        ones_sb[:, 0:512],
                        start=True,
                        stop=False,
                    )
                    for m in range(NM):
                        nc.tensor.matmul(
                            ps[:],
                            wk_sb[:, m * DKS + dkt * 128 : m * DKS + dkt * 128 + 128],
                            xT_sb[:, m * S + st * 512 : m * S + st * 512 + 512],
                            start=False,
                            stop=(m == NM - 1),
                        )
                    nc.scalar.copy(
                        kt_sb[:, dkt * S + st * 512 : dkt * S + st * 512 + 512], ps[:]
                    )

            # V[s, dv] with a ones column per head (col 256 of each strip)
            nc.vector.memset(
                v_sb[:].rearrange("p (t h c) -> p t h c", t=NST, h=HPC)[:, :, :, DV],
                1.0,
            )
            for st in range(NST):
                for dvh in range(2):  # dv halves of 512 = heads (2*dvh, 2*dvh+1)
                    ps = psP.tile([128, 512], F32, tag="proj")
                    nc.tensor.matmul(
                        ps[:],
                        ones_sb[:, 0:128],
                        bv_sb[:, dvh * 512 : dvh * 512 + 512],
                        start=True,
                        stop=False,
                    )
                    for m in range(NM):
                        nc.tensor.matmul(
                            ps[:],
                            xT_sb[:, m * S + st * 128 : m * S + st * 128 + 128],
                            wv_sb[:, m * DVS + dvh * 512 : m * DVS + dvh * 512 + 512],
                            start=False,
                            stop=(m == NM - 1),
                        )
                    base = st * HPC * VSTRIDE
                    for hh in range(2):
                        h = 2 * dvh + hh
                        nc.scalar.copy(
                            v_sb[:, base + h * VSTRIDE : base + h * VSTRIDE + DV],
                            ps[:, hh * 256 : hh * 256 + 256],
                        )

        # ---- attention (head pairs, row-tiled scores) ----------------
        # scores^T[k, q]: contraction is dk=64, so heads 2p (PE rows 0-63)
        # and 2p+1 (rows 64-127) run concurrently via tile_position row
        # tiling.  AV groups run in default 128x128 mode afterwards;
        # exp without max-subtraction; o tiles transposed by xbar DMA.
        with (
            tc.tile_pool(name="expp", bufs=1) as expp,
            tc.tile_pool(name="otile", bufs=4) as otp,
            tc.tile_pool(name="psSt", bufs=3, space="PSUM") as psS,
            tc.tile_pool(name="psAv", bufs=2, space="PSUM") as psV,
        ):
            NQT2 = S // 1024
            for pair in range(2):
                co = pair * S           # both heads of the pair share chunk co

                def st_tile(j, kt, hl, exps):
                    po = 64 * (hl % 2)
                    t = kt - 8 * j   # >=0 on diagonal k-tiles
                    toff = max(t, 0) * 128
                    q0 = j * 1024 + toff
                    ps = psS.tile([128, 1024], F32, tag="st")
                    lo_w = max(0, 512 - toff)
                    if lo_w:
                        nc.tensor.matmul(
                            ps[:, toff : toff + lo_w],
                            kt_sb[po : po + 64, co + kt * 128 : co + kt * 128 + 128],
                            qT_sb[po : po + 64, co + q0 : co + q0 + lo_w],
                            start=True,
                            stop=True,
                            tile_position=(po, 0),
                        )
                    nc.tensor.matmul(
                        ps[:, max(toff, 512) : 1024],
                        kt_sb[po : po + 64, co + kt * 128 : co + kt * 128 + 128],
                        qT_sb[po : po + 64, co + j * 1024 + max(toff, 512) : co + (j + 1) * 1024],
                        start=True,
                        stop=True,
                        tile_position=(po, 0),
                    )
                    nc.scalar.activation(
                        exps[:, kt * 1024 + toff : (kt + 1) * 1024],
                        ps[:, toff:1024],
                        AF.Exp,
                    )
                    if t >= 0:  # mask the diagonal 128x128 block
                        blk = exps[:, kt * 1024 + toff : kt * 1024 + toff + 128]
                        nc.vector.tensor_mul(blk, blk, mask_sb[:])

                def av_tile(j, sq, hl, exps):
                    i = 8 * j + sq
                    pso = psV.tile([128, VSTRIDE], F32, tag="av")
                    for kt in range(i + 1):
                        vb = kt * HPC * VSTRIDE + hl * VSTRIDE
                        nc.tensor.matmul(
                            pso[:],
                            exps[:, kt * 1024 + sq * 128 : kt * 1024 + sq * 128 + 128],
                            v_sb[:, vb : vb + VSTRIDE],
                            start=(kt == 0),
                            stop=(kt == i),
                        )
                    recip = smallp.tile([128, 1], F32, tag="recip")
                    nc.vector.reciprocal(recip[:], pso[:, DV : DV + 1])
                    ot = otp.tile([128, DV], BF16, tag="ot")
                    nc.vector.tensor_scalar_mul(ot[:], pso[:, 0:DV], recip[:])
                    for half in range(2):
                        hc = 2 * hl + half
                        nc.sync.dma_start_transpose(
                            got_sb[:, hc * S + i * 128 : hc * S + i * 128 + 128],
                            ot[:, half * 128 : half * 128 + 128],
                        )

                for j in range(NQT2):   # 1024-wide q tiles
                    hA, hB = 2 * pair, 2 * pair + 1
                    exps_a = expp.tile([128, 16 * 1024], BF16, tag="expSA")
                    exps_b = expp.tile([128, 16 * 1024], BF16, tag="expSB")
                    # row-tiled score phase: both heads stream concurrently
                    for kt in range(8 * j + 8):
                        st_tile(j, kt, hA, exps_a)
                        st_tile(j, kt, hB, exps_b)
                    # default-mode AV phase
                    for sq in range(8):
                        av_tile(j, sq, hA, exps_a)
                        av_tile(j, sq, hB, exps_b)

        # ---- gelu (exact erf) in place on transposed layout ----------
        for hc in range(NM):
            nc.scalar.activation(
                got_sb[:, hc * S : (hc + 1) * S],
                got_sb[:, hc * S : (hc + 1) * S],
                AF.Gelu,
            )

        # ---- FF partial + chunked ReduceScatter + gpsimd residual ----
        with (
            tc.tile_pool(name="ffw", bufs=1) as ffwp,
            tc.tile_pool(name="ffout", bufs=4) as ffoutp,
            tc.tile_pool(name="psFf", bufs=3, space="PSUM") as psF,
        ):
            wf_sb = load_chunked(ffwp, wf, D)
            for g in range(4):
                partial_d = dramp.tile([512, D], BF16, tag=f"part{g}")
                for cc in range(4):
                    c = 4 * g + cc
                    ps0 = psF.tile([128, 512], F32, tag="ff0")
                    ps1 = psF.tile([128, 512], F32, tag="ff1")
                    for hc in range(NM):
                        lhsT = got_sb[:, hc * S + c * 128 : hc * S + c * 128 + 128]
                        nc.tensor.matmul(
                            ps0[:], lhsT, wf_sb[:, hc * D : hc * D + 512],
                            start=(hc == 0), stop=(hc == NM - 1),
                        )
                        nc.tensor.matmul(
                            ps1[:], lhsT, wf_sb[:, hc * D + 512 : hc * D + 1024],
                            start=(hc == 0), stop=(hc == NM - 1),
                        )
                    fo = ffoutp.tile([128, D], BF16, tag="ffout")
                    nc.vector.tensor_copy(fo[:, 0:512], ps0[:])
                    nc.vector.tensor_copy(fo[:, 512:1024], ps1[:])
                    nc.scalar.dma_start(partial_d[cc * 128 : (cc + 1) * 128, :], fo[:])
                rs_d = dramp.tile([128, D], BF16, tag=f"rs{g}")
                nc.gpsimd.collective_compute(
                    "ReduceScatter",
                    mybir.AluOpType.add,
                    replica_groups=[[0, 1, 2, 3], [4, 5, 6, 7]],
                    ins=[partial_d.opt()],
                    outs=[rs_d.opt()],
                )
                # residual: RS-gated cast-DMA on the GpSimd queue (ordered
                # behind this RS), add on DVE, store on ACT
                rf = rfp.tile([128, D], F32, tag="rf")
                nc.gpsimd.dma_start(rf[:], rs_d[:])
                nc.vector.tensor_add(xrs[g][:], xrs[g][:], rf[:])
                nc.scalar.dma_start(out[g * 128 : (g + 1) * 128, :], xrs[g][:])


def make_in_maps(x, Wk, bk, Wv, bv, Wf, bf):
    """Host-side sharding: returns the per-core input dict list."""
    x = np.asarray(x, np.float32)
    Wk = np.asarray(Wk, np.float32)
    Wv = np.asarray(Wv, np.float32)
    Wf = np.asarray(Wf, np.float32)
    bk = np.asarray(bk, np.float32)
    bv = np.asarray(bv, np.float32)
    bf = np.asarray(bf, np.float32)
    mask = np.tril(np.ones((128, 128), np.float32)).T  # mask[k,q]=1 iff k<=q
    in_maps = []
    for c in range(NCORES):
        b, r = c // GROUP, c % GROUP
        xb = x[b]                                    # [S, D]
        xT = np.ascontiguousarray(xb.T).astype(bf16)
        qTs = xT[DKS * r : DKS * (r + 1)]            # heads 4r..4r+3 rows
        # chunked RS: core (b,r) tile g holds x rows 512g+128r+[0,128)
        xres = np.concatenate(
            [xb[512 * g + 128 * r : 512 * g + 128 * r + 128] for g in range(4)]
        ) + bf[None, :].astype(np.float32)
        in_maps.append({
            "xT": xT,
            "qT": np.ascontiguousarray(qTs),
            "xres": np.ascontiguousarray(xres),
            "wk": np.ascontiguousarray(Wk[:, DKS * r : DKS * (r + 1)]).astype(bf16),
            "wv": np.ascontiguousarray(Wv[:, DVS * r : DVS * (r + 1)]).astype(bf16),
            "wf": np.ascontiguousarray(Wf[DVS * r : DVS * (r + 1), :]).astype(bf16),
            "bkb": bk[None, DKS * r : DKS * (r + 1)].astype(bf16),
            "bvb": bv[None, DVS * r : DVS * (r + 1)].astype(bf16),
            "maskt": mask.astype(bf16),
            "ident": np.eye(128, dtype=np.float32).astype(bf16),
            "onesr": np.ones((1, 512), bf16),
        })
    return in_maps


def assemble(results):
    """[8 x [512,1024]] core outputs -> [2,2048,1024]."""
    out = np.empty((B, S, D), np.float32)
    for c in range(NCORES):
        b, r = c // GROUP, c % GROUP
        for g in range(4):
            out[b, 512 * g + 128 * r : 512 * g + 128 * r + 128, :] = results[c][
                "out"
            ][128 * g : 128 * (g + 1)]
    return out


def kernel(x, Wk, bk, Wv, bv, Wf, bf, _trace=False, _trace_cores=None):
    global _compiled
    if _compiled is None:
        _compiled = build_program()
    nc = _compiled
    in_maps = make_in_maps(x, Wk, bk, Wv, bv, Wf, bf)
    res = bass_utils.run_bass_kernel_spmd(
        nc,
        in_maps,
        core_ids=list(range(NCORES)),
        trace=_trace,
        trace_cores=_trace_cores,
    )
    out = assemble(res.results)
    kernel.last_result = res
    return out



# revision 5
# speedup vs baseline: 1.0321x; 1.0321x over previous
"""Trainium2 Bass kernel for nn_ExperimentalLayer9 (dense transformer layer).

Layer: x + gelu(attn(x)) @ Wf with
  Q = split_heads(x), K = split_heads(x@Wk+bk), V = split_heads(x@Wv+bv)
  causal softmax (no 1/sqrt(d) scale), exact-erf gelu, residual add.

Sharding over 8 NeuronCores: 2 batch groups x 4-way head/tensor parallel.
Core c handles batch b=c//4 and heads [4r, 4r+4) with r=c%4.

v2 schedule (vs the earlier baseline):
 - q-block-major ("j-major") attention: scores/exp/AV run per (head,
   512-wide q block), so the FF partial for a finished q block and its
   ReduceScatter launch while later attention blocks still compute.
   The RS tail shrinks from ~130us to the last 256-row chunk.
 - FF matmuls run in fp8 (e4m3, DoubleRow perf mode, 2x throughput):
   gelu output and Wf are cast to fp8 (all values are << e4m3 max, no
   scaling needed; measured rel-err ~1.3e-2 < 2e-2 gate).
 - RS in 8 chunks of 256 rows; residual add per chunk right after.
 - SBUF slot sharing: xT halves are reused for got8/xres, wk for wf8.

All other matmuls run in bf16 (fp32 PSUM accumulation); softmax in
fp32/bf16.  exp without max-subtraction (scores are bounded); the
exp-sum l(q) comes free from a ones-column appended to V.
"""

import numpy as np
import ml_dtypes

import concourse.bass as bass
import concourse.mybir as mybir
import concourse.tile as tile
from concourse import bacc
from concourse import bass_utils

# Problem shapes (hardcoded per contest contract).
B, S, D, H, DHID = 2, 2048, 1024, 16, 4096
NCORES = 8
GROUP = 4              # cores per batch group
HPC = 4                # heads per core
DK = 64                # q/k head dim
DV = 256               # v head dim
DKS = HPC * DK         # 256  k-slice per core
DVS = HPC * DV         # 1024 v/hidden slice per core
NM = D // 128          # 8    contraction chunks over d_model
NST = S // 128         # 16   s chunks of 128
VSTRIDE = DV + 1       # 257  V columns per head incl. ones column
NG = 8                 # RS chunks (256 rows each)
GR = S // NG           # 256  rows per RS chunk
ORC = GR // GROUP      # 64   output rows per core per chunk
ROWS = S // GROUP      # 512  output rows per core total

BF16 = mybir.dt.bfloat16
F32 = mybir.dt.float32
FP8 = mybir.dt.float8e4
AF = mybir.ActivationFunctionType
DR = mybir.MatmulPerfMode.DoubleRow

bf16 = ml_dtypes.bfloat16
f8 = ml_dtypes.float8_e4m3

_compiled = None


def build_program():
    nc = bacc.Bacc(
        "TRN2",
        target_bir_lowering=False,
        debug=False,
        enable_asserts=True,
        num_devices=NCORES,
    )

    # Per-core inputs (values differ per core; program is SPMD-identical).
    xT = nc.dram_tensor("xT", [D, S], BF16, kind="ExternalInput").ap()
    qT = nc.dram_tensor("qT", [DKS, S], BF16, kind="ExternalInput").ap()
    xres = nc.dram_tensor("xres", [ROWS, D], F32, kind="ExternalInput").ap()
    wk = nc.dram_tensor("wk", [D, DKS], BF16, kind="ExternalInput").ap()
    wv = nc.dram_tensor("wv", [D, DVS], BF16, kind="ExternalInput").ap()
    wf8 = nc.dram_tensor("wf8", [DVS, D], FP8, kind="ExternalInput").ap()
    bkb = nc.dram_tensor("bkb", [1, DKS], BF16, kind="ExternalInput").ap()
    bvb = nc.dram_tensor("bvb", [1, DVS], BF16, kind="ExternalInput").ap()
    maskt = nc.dram_tensor("maskt", [128, 128], BF16, kind="ExternalInput").ap()
    onesr = nc.dram_tensor("onesr", [1, 512], BF16, kind="ExternalInput").ap()
    out = nc.dram_tensor("out", [ROWS, D], F32, kind="ExternalOutput").ap()

    with tile.TileContext(nc) as tc:
        _body(nc, tc, xT, qT, xres, wk, wv, wf8, bkb, bvb, maskt, onesr, out)

    nc.compile()
    return nc


def _body(nc, tc, xT, qT, xres, wk, wv, wf8, bkb, bvb, maskt, onesr, out):
    with (
        tc.tile_pool(name="const", bufs=1) as constp,
        tc.tile_pool(name="kv", bufs=1) as kvp,
        tc.tile_pool(name="sh1", bufs=1) as sh1p,   # xT_lo <-> got8
        tc.tile_pool(name="sh2", bufs=1) as sh2p,   # xT_hi <-> xres
        tc.tile_pool(name="sh3", bufs=1) as sh3p,   # wk <-> wf8
        tc.tile_pool(name="wvp", bufs=1) as wvp,
        tc.tile_pool(name="expp", bufs=2) as expp,
        tc.tile_pool(name="gotp", bufs=1) as gotp,
        tc.tile_pool(name="small", bufs=4) as smallp,
        tc.tile_pool(name="ffout", bufs=4) as ffoutp,
        tc.tile_pool(name="rfp", bufs=2) as rfp,
        tc.tile_pool(name="psS", bufs=3, space="PSUM") as psS,
        tc.tile_pool(name="psV", bufs=2, space="PSUM") as psV,
        tc.tile_pool(name="psG", bufs=3, space="PSUM") as psG,
        tc.tile_pool(name="dram", bufs=1, space="DRAM") as dramp,
    ):
        # ---- constants (ACT queue) ------------------------------------
        ones_sb = constp.tile([1, 512], BF16)
        nc.scalar.dma_start(ones_sb[:], onesr[:])
        mask_sb = constp.tile([128, 128], BF16)
        nc.scalar.dma_start(mask_sb[:], maskt[:])
        bk_sb = constp.tile([1, DKS], BF16)
        nc.scalar.dma_start(bk_sb[:], bkb[:])
        bv_sb = constp.tile([1, DVS], BF16)
        nc.scalar.dma_start(bv_sb[:], bvb[:])

        # Warm up the collectives path (ncfw/channel setup) so the first
        # real ReduceScatter doesn't pay ~25us of first-call overhead.
        warm_in = dramp.tile([4, 16], BF16, tag="warm_in")
        warm_out = dramp.tile([1, 16], BF16, tag="warm_out")
        nc.scalar.dma_start(
            warm_in[:].rearrange("a b -> (a b)")[None, :], ones_sb[0:1, 0:64]
        )
        nc.gpsimd.collective_compute(
            "ReduceScatter",
            mybir.AluOpType.add,
            replica_groups=[[0, 1, 2, 3], [4, 5, 6, 7]],
            ins=[warm_in.opt()],
            outs=[warm_out.opt()],
        )

        # ---- input loads (Sync queue; K-proj deps first) --------------
        wk_sb = sh3p.tile([128, NM, DKS], BF16, tag="sh3t")
        for m in range(NM):
            nc.sync.dma_start(wk_sb[:, m, :], wk[m * 128 : (m + 1) * 128, :])
        xT_lo = sh1p.tile([128, NM, 1024], BF16, tag="sh1t")
        xT_hi = sh2p.tile([128, NM, 1024], BF16, tag="sh2t")
        for m in range(NM):
            nc.sync.dma_start(xT_lo[:, m, :], xT[m * 128 : (m + 1) * 128, 0:1024])
        qT_sb = kvp.tile([128, 2, S], BF16)
        for dkt in range(2):
            nc.sync.dma_start(qT_sb[:, dkt, :], qT[dkt * 128 : (dkt + 1) * 128, :])
        for m in range(NM):
            nc.sync.dma_start(xT_hi[:, m, :], xT[m * 128 : (m + 1) * 128, 1024:2048])
        wv_sb = wvp.tile([128, NM, DVS], BF16)
        for m in range(NM):
            nc.sync.dma_start(wv_sb[:, m, :], wv[m * 128 : (m + 1) * 128, :])

        kt_sb = kvp.tile([128, 2, S], BF16)   # K^T rows dk%128, chunk dk//128
        v_sb = kvp.tile([128, NST, HPC, VSTRIDE], BF16)
        got_sb = gotp.tile([128, NM, S], BF16)  # o^T (later gelu'd), hc-major

        def xT_chunk(m, st):   # [128, 128] chunk of x^T: rows m, cols s-tile st
            t, off = (xT_lo, st) if st < 8 else (xT_hi, st - 8)
            return t[:, m, off * 128 : (off + 1) * 128]

        def xT_s512(m, sg):    # [128, 512] chunk: rows m, s-512-block sg
            t, off = (xT_lo, sg) if sg < 2 else (xT_hi, sg - 2)
            return t[:, m, off * 512 : (off + 1) * 512]

        # ---- K^T projection (st-major so scores can start early) ------
        for sg in range(4):
            for dkt in range(2):
                ps = psG.tile([128, 512], F32, tag="gen")
                nc.tensor.matmul(
                    ps[:], bk_sb[:, dkt * 128 : (dkt + 1) * 128],
                    ones_sb[:, 0:512], start=True, stop=False,
                )
                for m in range(NM):
                    nc.tensor.matmul(
                        ps[:],
                        wk_sb[:, m, dkt * 128 : dkt * 128 + 128],
                        xT_s512(m, sg),
                        start=False, stop=(m == NM - 1),
                    )
                nc.scalar.copy(kt_sb[:, dkt, sg * 512 : (sg + 1) * 512], ps[:])

        # ---- V projection ---------------------------------------------
        # v_sb[s, st, h, 0:256] = V values, col 256 = 1.0 (for the exp-sum)
        nc.vector.memset(v_sb[:, :, :, DV], 1.0)

        def v_proj(st_range):
            for st in st_range:
                for dvh in range(2):  # dv halves of 512 = heads (2dvh, 2dvh+1)
                    ps = psG.tile([128, 512], F32, tag="gen")
                    nc.tensor.matmul(
                        ps[:], ones_sb[:, 0:128],
                        bv_sb[:, dvh * 512 : dvh * 512 + 512],
                        start=True, stop=False,
                    )
                    for m in range(NM):
                        nc.tensor.matmul(
                            ps[:], xT_chunk(m, st),
                            wv_sb[:, m, dvh * 512 : dvh * 512 + 512],
                            start=False, stop=(m == NM - 1),
                        )
                    for hh in range(2):
                        nc.scalar.copy(
                            v_sb[:, st, 2 * dvh + hh, 0:DV],
                            ps[:, hh * 256 : hh * 256 + 256],
                        )

        # ---- attention pieces ------------------------------------------
        # scores^T[k, q] per (head, 512-q-block qg, 128-k-chunk kt), exp'd
        # into exps[:, kt, :]; causal diagonal masked by mask_sb multiply.
        def attn_block(h, qg, exps):
            po = 64 * (h % 2)
            dkt = h // 2
            q0 = qg * 512
            for kt in range(4 * qg + 4):
                t = kt - 4 * qg
                toff = max(t, 0) * 128
                ps = psS.tile([128, 512], F32, tag="st")
                nc.tensor.matmul(
                    ps[:, toff:512],
                    kt_sb[po : po + 64, dkt, kt * 128 : kt * 128 + 128],
                    qT_sb[po : po + 64, dkt, q0 + toff : q0 + 512],
                    start=True, stop=True,
                    tile_position=(po, 0),
                )
                nc.scalar.activation(
                    exps[:, kt, toff:512], ps[:, toff:512], AF.Exp
                )
                if t >= 0:
                    blk = exps[:, kt, toff : toff + 128]
                    nc.vector.tensor_mul(blk, blk, mask_sb[:])
            for sq in range(4):
                i = 4 * qg + sq
                pso = psV.tile([128, VSTRIDE], F32, tag="av")
                for kt in range(i + 1):
                    nc.tensor.matmul(
                        pso[:],
                        exps[:, kt, sq * 128 : sq * 128 + 128],
                        v_sb[:, kt, h, :],
                        start=(kt == 0), stop=(kt == i),
                    )
                recip = smallp.tile([128, 1], F32, tag="recip")
                nc.vector.reciprocal(recip[:], pso[:, DV : DV + 1])
                ot = smallp.tile([128, DV], BF16, tag="ot")
                nc.vector.tensor_scalar_mul(ot[:], pso[:, 0:DV], recip[:])
                for half in range(2):
                    hc = 2 * h + half
                    nc.sync.dma_start_transpose(
                        got_sb[:, hc, i * 128 : i * 128 + 128],
                        ot[:, half * 128 : half * 128 + 128],
                    )

        # ---- FF chunk (256 q rows) + RS + residual ---------------------
        def ff_chunk(g, got8, xres_sb):
            partial_d = dramp.tile([GR, D], BF16, tag=f"part{g}")
            for cc in range(2):
                c = 2 * g + cc
                ps0 = psG.tile([128, 512], F32, tag="gen")
                ps1 = psG.tile([128, 512], F32, tag="gen")
                for dc in range(4):
                    lhsT = got8[:, 2 * dc : 2 * dc + 2, c * 128 : c * 128 + 128]
                    nc.tensor.matmul(
                        ps0[:], lhsT, wf8_sb[:, 2 * dc : 2 * dc + 2, 0:512],
                        start=(dc == 0), stop=(dc == 3), perf_mode=DR,
                    )
                    nc.tensor.matmul(
                        ps1[:], lhsT, wf8_sb[:, 2 * dc : 2 * dc + 2, 512:1024],
                        start=(dc == 0), stop=(dc == 3), perf_mode=DR,
                    )
                fo = ffoutp.tile([128, D], BF16, tag="ffout")
                nc.vector.tensor_copy(fo[:, 0:512], ps0[:])
                nc.vector.tensor_copy(fo[:, 512:1024], ps1[:])
                nc.scalar.dma_start(partial_d[cc * 128 : (cc + 1) * 128, :], fo[:])
            rs_d = dramp.tile([ORC, D], BF16, tag=f"rs{g}")
            nc.gpsimd.collective_compute(
                "ReduceScatter",
                mybir.AluOpType.add,
                replica_groups=[[0, 1, 2, 3], [4, 5, 6, 7]],
                ins=[partial_d.opt()],
                outs=[rs_d.opt()],
            )
            # residual: RS-gated cast-DMA on the GpSimd queue (ordered
            # behind this RS), add on DVE, store on ACT
            po = 64 * (g % 2)
            rf = rfp.tile([128, D], F32, tag="rf")
            nc.gpsimd.dma_start(rf[po : po + ORC, :], rs_d[:])
            xr = xres_sb[po : po + ORC, g // 2, :]
            nc.vector.tensor_add(xr, xr, rf[po : po + ORC, :])
            nc.scalar.dma_start(out[g * ORC : (g + 1) * ORC, :], xr)

        def gelu_chunks(j):   # cast gelu(o^T) -> fp8 for the 1024-q block j
            for hc in range(NM):
                nc.scalar.activation(
                    got8[:, hc, j * 1024 : (j + 1) * 1024],
                    got_sb[:, hc, j * 1024 : (j + 1) * 1024],
                    AF.Gelu,
                )

        # ================= schedule =====================================
        v_proj(range(8))          # V for k < 1024 (feeds attention j=0)

        for h in range(HPC):      # attention q < 1024
            for qg in range(2):
                exps = expp.tile([128, 16, 512], BF16, tag="exps")
                attn_block(h, qg, exps)

        v_proj(range(8, 16))      # V for k >= 1024 (overlaps j=0 tail)

        # xT dead now: load wf8 / xres into the shared slots (gpsimd queue)
        wf8_sb = sh3p.tile([128, NM, D], FP8, tag="sh3t")
        for m in range(NM):
            nc.gpsimd.dma_start(wf8_sb[:, m, :], wf8[m * 128 : (m + 1) * 128, :])
        got8 = sh1p.tile([128, NM, S], FP8, tag="sh1t")
        xres_sb = sh2p.tile([128, 4, D], F32, tag="sh2t")
        for g4 in range(4):
            nc.gpsimd.dma_start(
                xres_sb[:, g4, :], xres[g4 * 128 : (g4 + 1) * 128, :]
            )

        # scores for j=1 can proceed while j=0 transposes/gelu drain
        for h in range(2):        # heads 0,1: q-block qg=2 scores+AV
            exps = expp.tile([128, 16, 512], BF16, tag="exps")
            attn_block(h, 2, exps)

        gelu_chunks(0)            # gelu+fp8 cast for q < 1024
        for g in range(4):        # FF + RS for q < 1024 (overlaps attn j=1)
            ff_chunk(g, got8, xres_sb)

        for h in range(2):
            exps = expp.tile([128, 16, 512], BF16, tag="exps")
            attn_block(h, 3, exps)
        for h in range(2, HPC):
            for qg in range(2, 4):
                exps = expp.tile([128, 16, 512], BF16, tag="exps")
                attn_block(h, qg, exps)

        gelu_chunks(1)
        for g in range(4, NG):
            ff_chunk(g, got8, xres_sb)


def make_in_maps(x, Wk, bk, Wv, bv, Wf, bf):
    """Host-side sharding: returns the per-core input dict list."""
    x = np.asarray(x, np.float32)
    Wk = np.asarray(Wk, np.float32)
    Wv = np.asarray(Wv, np.float32)
    Wf = np.asarray(Wf, np.float32)
    bk = np.asarray(bk, np.float32)
    bv = np.asarray(bv, np.float32)
    bf = np.asarray(bf, np.float32)
    mask = np.tril(np.ones((128, 128), np.float32)).T  # mask[k,q]=1 iff k<=q
    in_maps = []
    for c in range(NCORES):
        b, r = c // GROUP, c % GROUP
        xb = x[b]                                    # [S, D]
        xT = np.ascontiguousarray(xb.T).astype(bf16)
        qTs = xT[DKS * r : DKS * (r + 1)]            # heads 4r..4r+3 rows
        # chunked RS: core (b,r) chunk g holds x rows 256g+64r+[0,64)
        xres = np.concatenate(
            [xb[GR * g + ORC * r : GR * g + ORC * r + ORC] for g in range(NG)]
        ) + bf[None, :].astype(np.float32)
        in_maps.append({
            "xT": xT,
            "qT": np.ascontiguousarray(qTs),
            "xres": np.ascontiguousarray(xres),
            "wk": np.ascontiguousarray(Wk[:, DKS * r : DKS * (r + 1)]).astype(bf16),
            "wv": np.ascontiguousarray(Wv[:, DVS * r : DVS * (r + 1)]).astype(bf16),
            "wf8": np.ascontiguousarray(Wf[DVS * r : DVS * (r + 1), :]).astype(f8),
            "bkb": bk[None, DKS * r : DKS * (r + 1)].astype(bf16),
            "bvb": bv[None, DVS * r : DVS * (r + 1)].astype(bf16),
            "maskt": mask.astype(bf16),
            "onesr": np.ones((1, 512), bf16),
        })
    return in_maps


def assemble(results):
    """[8 x [512,1024]] core outputs -> [2,2048,1024]."""
    out = np.empty((B, S, D), np.float32)
    for c in range(NCORES):
        b, r = c // GROUP, c % GROUP
        for g in range(NG):
            out[b, GR * g + ORC * r : GR * g + ORC * r + ORC, :] = results[c][
                "out"
            ][ORC * g : ORC * (g + 1)]
    return out


def kernel(x, Wk, bk, Wv, bv, Wf, bf, _trace=False, _trace_cores=None):
    global _compiled
    if _compiled is None:
        _compiled = build_program()
    nc = _compiled
    in_maps = make_in_maps(x, Wk, bk, Wv, bv, Wf, bf)
    res = bass_utils.run_bass_kernel_spmd(
        nc,
        in_maps,
        core_ids=list(range(NCORES)),
        trace=_trace,
        trace_cores=_trace_cores,
    )
    out = assemble(res.results)
    kernel.last_result = res
    return out


# revision 9
# speedup vs baseline: 1.1355x; 1.1002x over previous
"""Trainium2 Bass kernel for nn_ExperimentalLayer9 (dense transformer layer).

Layer: x + gelu(attn(x)) @ Wf with
  Q = split_heads(x), K = split_heads(x@Wk+bk), V = split_heads(x@Wv+bv)
  causal softmax (no 1/sqrt(d) scale), exact-erf gelu, residual add.

Sharding over 8 NeuronCores: 2 batch groups x 4-way head/tensor parallel.
Core c handles batch b=c//4 and heads [4r, 4r+4) with r=c%4.

v2 schedule (vs the earlier baseline):
 - q-block-major ("j-major") attention: scores/exp/AV run per (head,
   512-wide q block), so the FF partial for a finished q block and its
   ReduceScatter launch while later attention blocks still compute.
   The RS tail shrinks from ~130us to the last 256-row chunk.
 - FF matmuls run in fp8 (e4m3, DoubleRow perf mode, 2x throughput):
   gelu output and Wf are cast to fp8 (all values are << e4m3 max, no
   scaling needed; measured rel-err ~1.3e-2 < 2e-2 gate).
 - RS in 8 chunks of 256 rows; residual add per chunk right after.
 - SBUF slot sharing: xT halves are reused for got8/xres, wk for wf8.

All other matmuls run in bf16 (fp32 PSUM accumulation); softmax in
fp32/bf16.  exp without max-subtraction (scores are bounded); the
exp-sum l(q) comes free from a ones-column appended to V.
"""

import numpy as np
import ml_dtypes

import concourse.bass as bass
import concourse.mybir as mybir
import concourse.tile as tile
from concourse import bacc
from concourse import bass_utils

# Problem shapes (hardcoded per contest contract).
B, S, D, H, DHID = 2, 2048, 1024, 16, 4096
NCORES = 8
GROUP = 4              # cores per batch group
HPC = 4                # heads per core
DK = 64                # q/k head dim
DV = 256               # v head dim
DKS = HPC * DK         # 256  k-slice per core
DVS = HPC * DV         # 1024 v/hidden slice per core
NM = D // 128          # 8    contraction chunks over d_model
NST = S // 128         # 16   s chunks of 128
VSTRIDE = DV + 1       # 257  V columns per head incl. ones column
NG = 8                 # RS chunks (256 rows each)
GR = S // NG           # 256  rows per RS chunk
ORC = GR // GROUP      # 64   output rows per core per chunk
ROWS = S // GROUP      # 512  output rows per core total

BF16 = mybir.dt.bfloat16
F32 = mybir.dt.float32
FP8 = mybir.dt.float8e4
AF = mybir.ActivationFunctionType
DR = mybir.MatmulPerfMode.DoubleRow

bf16 = ml_dtypes.bfloat16
f8 = ml_dtypes.float8_e4m3

_compiled = None


def build_program():
    nc = bacc.Bacc(
        "TRN2",
        target_bir_lowering=False,
        debug=False,
        enable_asserts=True,
        num_devices=NCORES,
    )

    # Per-core inputs (values differ per core; program is SPMD-identical).
    xT = nc.dram_tensor("xT", [D, S], BF16, kind="ExternalInput").ap()
    qT = nc.dram_tensor("qT", [DKS, S], BF16, kind="ExternalInput").ap()
    xres = nc.dram_tensor("xres", [ROWS, D], F32, kind="ExternalInput").ap()
    wk = nc.dram_tensor("wk", [D, DKS], BF16, kind="ExternalInput").ap()
    wv = nc.dram_tensor("wv", [D, DVS], BF16, kind="ExternalInput").ap()
    wf8 = nc.dram_tensor("wf8", [DVS, D], FP8, kind="ExternalInput").ap()
    bkb = nc.dram_tensor("bkb", [1, DKS], BF16, kind="ExternalInput").ap()
    bvb = nc.dram_tensor("bvb", [1, DVS], BF16, kind="ExternalInput").ap()
    maskt = nc.dram_tensor("maskt", [128, 128], BF16, kind="ExternalInput").ap()
    onesr = nc.dram_tensor("onesr", [1, 512], BF16, kind="ExternalInput").ap()
    out = nc.dram_tensor("out", [ROWS, D], F32, kind="ExternalOutput").ap()

    with tile.TileContext(nc) as tc:
        _body(nc, tc, xT, qT, xres, wk, wv, wf8, bkb, bvb, maskt, onesr, out)

    nc.compile()
    return nc


def _body(nc, tc, xT, qT, xres, wk, wv, wf8, bkb, bvb, maskt, onesr, out):
    with (
        tc.tile_pool(name="const", bufs=1) as constp,
        tc.tile_pool(name="kv", bufs=1) as kvp,
        tc.tile_pool(name="sh1", bufs=1) as sh1p,   # xT_lo <-> got8
        tc.tile_pool(name="sh2", bufs=1) as sh2p,   # xT_hi <-> xres
        tc.tile_pool(name="sh3", bufs=1) as sh3p,   # wk <-> wf8
        tc.tile_pool(name="wvp", bufs=1) as wvp,
        tc.tile_pool(name="expp", bufs=2) as expp,
        tc.tile_pool(name="gotp", bufs=1) as gotp,
        tc.tile_pool(name="small", bufs=4) as smallp,
        tc.tile_pool(name="ffout", bufs=4) as ffoutp,
        tc.tile_pool(name="rfp", bufs=2) as rfp,
        tc.tile_pool(name="psS", bufs=3, space="PSUM") as psS,
        tc.tile_pool(name="psV", bufs=2, space="PSUM") as psV,
        tc.tile_pool(name="psG", bufs=3, space="PSUM") as psG,
        tc.tile_pool(name="dram", bufs=1, space="DRAM") as dramp,
    ):
        # ---- constants (ACT queue) ------------------------------------
        ones_sb = constp.tile([1, 512], BF16)
        nc.scalar.dma_start(ones_sb[:], onesr[:])
        mask_sb = constp.tile([128, 128], BF16)
        nc.scalar.dma_start(mask_sb[:], maskt[:])
        bk_sb = constp.tile([1, DKS], BF16)
        nc.scalar.dma_start(bk_sb[:], bkb[:])
        bv_sb = constp.tile([1, DVS], BF16)
        nc.scalar.dma_start(bv_sb[:], bvb[:])

        # Warm up the collectives path (ncfw/channel setup) so the first
        # real ReduceScatter doesn't pay ~25us of first-call overhead.
        warm_in = dramp.tile([4, 16], BF16, tag="warm_in")
        warm_out = dramp.tile([1, 16], BF16, tag="warm_out")
        nc.scalar.dma_start(
            warm_in[:].rearrange("a b -> (a b)")[None, :], ones_sb[0:1, 0:64]
        )
        nc.gpsimd.collective_compute(
            "ReduceScatter",
            mybir.AluOpType.add,
            replica_groups=[[0, 1, 2, 3], [4, 5, 6, 7]],
            ins=[warm_in.opt()],
            outs=[warm_out.opt()],
        )

        # ---- input loads (Sync queue; K-proj deps first) --------------
        wk_sb = sh3p.tile([128, NM, DKS], BF16, tag="sh3t")
        for m in range(NM):
            nc.sync.dma_start(wk_sb[:, m, :], wk[m * 128 : (m + 1) * 128, :])
        xT_lo = sh1p.tile([128, NM, 1024], BF16, tag="sh1t")
        xT_hi = sh2p.tile([128, NM, 1024], BF16, tag="sh2t")
        for m in range(NM):
            nc.sync.dma_start(xT_lo[:, m, :], xT[m * 128 : (m + 1) * 128, 0:1024])
        qT_sb = kvp.tile([128, 2, S], BF16)
        for dkt in range(2):
            nc.sync.dma_start(qT_sb[:, dkt, :], qT[dkt * 128 : (dkt + 1) * 128, :])
        for m in range(NM):
            nc.sync.dma_start(xT_hi[:, m, :], xT[m * 128 : (m + 1) * 128, 1024:2048])
        wv_sb = wvp.tile([128, NM, DVS], BF16)
        for m in range(NM):
            nc.sync.dma_start(wv_sb[:, m, :], wv[m * 128 : (m + 1) * 128, :])

        kt_sb = kvp.tile([128, 2, S], BF16)   # K^T rows dk%128, chunk dk//128
        v_sb = kvp.tile([128, NST, HPC, VSTRIDE], BF16)
        got_sb = gotp.tile([128, NM, S], BF16)  # o^T (later gelu'd), hc-major

        def xT_chunk(m, st):   # [128, 128] chunk of x^T: rows m, cols s-tile st
            t, off = (xT_lo, st) if st < 8 else (xT_hi, st - 8)
            return t[:, m, off * 128 : (off + 1) * 128]

        def xT_s512(m, sg):    # [128, 512] chunk: rows m, s-512-block sg
            t, off = (xT_lo, sg) if sg < 2 else (xT_hi, sg - 2)
            return t[:, m, off * 512 : (off + 1) * 512]

        # ---- K^T projection (st-major so scores can start early) ------
        for sg in range(4):
            for dkt in range(2):
                ps = psG.tile([128, 512], F32, tag="gen")
                nc.tensor.matmul(
                    ps[:], bk_sb[:, dkt * 128 : (dkt + 1) * 128],
                    ones_sb[:, 0:512], start=True, stop=False,
                )
                for m in range(NM):
                    nc.tensor.matmul(
                        ps[:],
                        wk_sb[:, m, dkt * 128 : dkt * 128 + 128],
                        xT_s512(m, sg),
                        start=False, stop=(m == NM - 1),
                    )
                nc.scalar.copy(kt_sb[:, dkt, sg * 512 : (sg + 1) * 512], ps[:])

        # ---- V projection ---------------------------------------------
        # v_sb[s, st, h, 0:256] = V values, col 256 = 1.0 (for the exp-sum)
        nc.vector.memset(v_sb[:, :, :, DV], 1.0)

        def v_proj(st_range):
            for st in st_range:
                for dvh in range(2):  # dv halves of 512 = heads (2dvh, 2dvh+1)
                    ps = psG.tile([128, 512], F32, tag="gen")
                    nc.tensor.matmul(
                        ps[:], ones_sb[:, 0:128],
                        bv_sb[:, dvh * 512 : dvh * 512 + 512],
                        start=True, stop=False,
                    )
                    for m in range(NM):
                        nc.tensor.matmul(
                            ps[:], xT_chunk(m, st),
                            wv_sb[:, m, dvh * 512 : dvh * 512 + 512],
                            start=False, stop=(m == NM - 1),
                        )
                    for hh in range(2):
                        nc.scalar.copy(
                            v_sb[:, st, 2 * dvh + hh, 0:DV],
                            ps[:, hh * 256 : hh * 256 + 256],
                        )

        # ---- attention pieces ------------------------------------------
        # scores^T[k, q] per (head, 512-q-block qg, 128-k-chunk kt), exp'd
        # into exps[:, kt, :]; causal diagonal masked by mask_sb multiply.
        def scores_block(h, qg, exps):
            po = 64 * (h % 2)
            dkt = h // 2
            q0 = qg * 512
            for kt in range(4 * qg + 4):
                t = kt - 4 * qg
                toff = max(t, 0) * 128
                ps = psS.tile([128, 512], F32, tag="st")
                nc.tensor.matmul(
                    ps[:, toff:512],
                    kt_sb[po : po + 64, dkt, kt * 128 : kt * 128 + 128],
                    qT_sb[po : po + 64, dkt, q0 + toff : q0 + 512],
                    start=True, stop=True,
                    tile_position=(po, 0),
                )
                nc.scalar.activation(
                    exps[:, kt, toff:512], ps[:, toff:512], AF.Exp
                )
                if t >= 0:
                    blk = exps[:, kt, toff : toff + 128]
                    nc.vector.tensor_mul(blk, blk, mask_sb[:])

        def av_block(h, qg, exps):
            for sq in range(4):
                i = 4 * qg + sq
                pso = psV.tile([128, VSTRIDE], F32, tag="av")
                for kt in range(i + 1):
                    nc.tensor.matmul(
                        pso[:],
                        exps[:, kt, sq * 128 : sq * 128 + 128],
                        v_sb[:, kt, h, :],
                        start=(kt == 0), stop=(kt == i),
                    )
                recip = smallp.tile([128, 1], F32, tag="recip")
                nc.vector.reciprocal(recip[:], pso[:, DV : DV + 1])
                ot = smallp.tile([128, DV], BF16, tag="ot")
                nc.vector.tensor_scalar_mul(ot[:], pso[:, 0:DV], recip[:])
                for half in range(2):
                    hc = 2 * h + half
                    nc.sync.dma_start_transpose(
                        got_sb[:, hc, i * 128 : i * 128 + 128],
                        ot[:, half * 128 : half * 128 + 128],
                    )

        # ---- FF chunk (256 q rows) + RS + residual ---------------------
        def ff_chunk(g, got8, xres_sb):
            partial_d = dramp.tile([GR, D], BF16, tag=f"part{g}")
            for cc in range(2):
                c = 2 * g + cc
                ps0 = psG.tile([128, 512], F32, tag="gen")
                ps1 = psG.tile([128, 512], F32, tag="gen")
                for dc in range(4):
                    lhsT = got8[:, 2 * dc : 2 * dc + 2, c * 128 : c * 128 + 128]
                    nc.tensor.matmul(
                        ps0[:], lhsT, wf8_sb[:, 2 * dc : 2 * dc + 2, 0:512],
                        start=(dc == 0), stop=(dc == 3), perf_mode=DR,
                    )
                    nc.tensor.matmul(
                        ps1[:], lhsT, wf8_sb[:, 2 * dc : 2 * dc + 2, 512:1024],
                        start=(dc == 0), stop=(dc == 3), perf_mode=DR,
                    )
                fo = ffoutp.tile([128, D], BF16, tag="ffout")
                nc.vector.tensor_copy(fo[:, 0:512], ps0[:])
                nc.vector.tensor_copy(fo[:, 512:1024], ps1[:])
                nc.scalar.dma_start(partial_d[cc * 128 : (cc + 1) * 128, :], fo[:])
            rs_d = dramp.tile([ORC, D], BF16, tag=f"rs{g}")
            nc.gpsimd.collective_compute(
                "ReduceScatter",
                mybir.AluOpType.add,
                replica_groups=[[0, 1, 2, 3], [4, 5, 6, 7]],
                ins=[partial_d.opt()],
                outs=[rs_d.opt()],
            )
            return rs_d

        def residual(g, rs_d, xres_sb):
            # RS-gated cast-DMA on the GpSimd queue, add on DVE, store
            # on ACT.  Deferred to the end of the program so the blocking
            # wait on the RS never stalls mid-kernel DVE/ACT work.
            po = 64 * (g % 2)
            rf = rfp.tile([128, D], F32, tag="rf")
            nc.gpsimd.dma_start(rf[po : po + ORC, :], rs_d[:])
            xr = xres_sb[po : po + ORC, g // 2, :]
            nc.vector.tensor_add(xr, xr, rf[po : po + ORC, :])
            nc.scalar.dma_start(out[g * ORC : (g + 1) * ORC, :], xr)

        def gelu_chunk(qg):   # cast gelu(o^T) -> fp8 for 512-q-block qg
            for hc in range(NM):
                nc.scalar.activation(
                    got8[:, hc, qg * 512 : (qg + 1) * 512],
                    got_sb[:, hc, qg * 512 : (qg + 1) * 512],
                    AF.Gelu,
                )

        def new_exps():
            exps = expp.tile([128, 16, 512], BF16, tag="exps")
            return exps

        # ================= schedule =====================================
        # Per 512-q-block qg: attention for all heads, then gelu+FF+RS for
        # its two 256-row chunks while the next block's attention runs.
        # Tensor-queue filler (next block's h0 scores / late V projection)
        # covers the o^T-transpose latency before each FF.
        v_proj(range(8))          # V for k < 1024 (feeds qg 0,1)

        # wk dead after K proj: load wf8 into the shared slot (gpsimd q)
        wf8_sb = sh3p.tile([128, NM, D], FP8, tag="sh3t")
        for m in range(NM):
            nc.gpsimd.dma_start(wf8_sb[:, m, :], wf8[m * 128 : (m + 1) * 128, :])
        got8 = sh1p.tile([128, NM, S], FP8, tag="sh1t")
        xres_sb = sh2p.tile([128, 4, D], F32, tag="sh2t")

        rs_ds = [None] * NG

        def do_ff(qg):
            for g in (2 * qg, 2 * qg + 1):
                rs_ds[g] = ff_chunk(g, got8, xres_sb)

        # ---- qg0 ----
        e0 = new_exps()
        scores_block(0, 0, e0)
        av_block(0, 0, e0)
        for h in range(1, HPC):
            e = new_exps()
            scores_block(h, 0, e)
            av_block(h, 0, e)
        e0 = new_exps()
        scores_block(0, 1, e0)     # prefetch next block's h0
        v_proj(range(8, 16))       # V for k >= 1024; covers qg0 transposes
        # xT dead now: xres into the shared slot (gpsimd queue)
        for g4 in range(4):
            nc.gpsimd.dma_start(
                xres_sb[:, g4, :], xres[g4 * 128 : (g4 + 1) * 128, :]
            )
        gelu_chunk(0)
        do_ff(0)

        # ---- qg1..qg3: FF of block qg-1 is issued mid-attention of qg,
        # where its gelu/transposes are long since complete.
        for qg in range(1, 4):
            av_block(0, qg, e0)
            for h in range(1, HPC):
                e = new_exps()
                scores_block(h, qg, e)
                if h == 1 and qg >= 2:
                    do_ff(qg - 1)
                if h == 3 and qg < 3:
                    e0 = new_exps()
                    scores_block(0, qg + 1, e0)
                av_block(h, qg, e)
            gelu_chunk(qg)
        do_ff(3)

        # ---- residuals (all RS waits concentrated at the end) ----------
        for g in range(NG):
            residual(g, rs_ds[g], xres_sb)


def make_in_maps(x, Wk, bk, Wv, bv, Wf, bf):
    """Host-side sharding: returns the per-core input dict list."""
    x = np.asarray(x, np.float32)
    Wk = np.asarray(Wk, np.float32)
    Wv = np.asarray(Wv, np.float32)
    Wf = np.asarray(Wf, np.float32)
    bk = np.asarray(bk, np.float32)
    bv = np.asarray(bv, np.float32)
    bf = np.asarray(bf, np.float32)
    mask = np.tril(np.ones((128, 128), np.float32)).T  # mask[k,q]=1 iff k<=q
    in_maps = []
    for c in range(NCORES):
        b, r = c // GROUP, c % GROUP
        xb = x[b]                                    # [S, D]
        xT = np.ascontiguousarray(xb.T).astype(bf16)
        qTs = xT[DKS * r : DKS * (r + 1)]            # heads 4r..4r+3 rows
        # chunked RS: core (b,r) chunk g holds x rows 256g+64r+[0,64)
        xres = np.concatenate(
            [xb[GR * g + ORC * r : GR * g + ORC * r + ORC] for g in range(NG)]
        ) + bf[None, :].astype(np.float32)
        in_maps.append({
            "xT": xT,
            "qT": np.ascontiguousarray(qTs),
            "xres": np.ascontiguousarray(xres),
            "wk": np.ascontiguousarray(Wk[:, DKS * r : DKS * (r + 1)]).astype(bf16),
            "wv": np.ascontiguousarray(Wv[:, DVS * r : DVS * (r + 1)]).astype(bf16),
            "wf8": np.ascontiguousarray(Wf[DVS * r : DVS * (r + 1), :]).astype(f8),
            "bkb": bk[None, DKS * r : DKS * (r + 1)].astype(bf16),
            "bvb": bv[None, DVS * r : DVS * (r + 1)].astype(bf16),
            "maskt": mask.astype(bf16),
            "onesr": np.ones((1, 512), bf16),
        })
    return in_maps


def assemble(results):
    """[8 x [512,1024]] core outputs -> [2,2048,1024]."""
    out = np.empty((B, S, D), np.float32)
    for c in range(NCORES):
        b, r = c // GROUP, c % GROUP
        for g in range(NG):
            out[b, GR * g + ORC * r : GR * g + ORC * r + ORC, :] = results[c][
                "out"
            ][ORC * g : ORC * (g + 1)]
    return out


def kernel(x, Wk, bk, Wv, bv, Wf, bf, _trace=False, _trace_cores=None):
    global _compiled
    if _compiled is None:
        _compiled = build_program()
    nc = _compiled
    in_maps = make_in_maps(x, Wk, bk, Wv, bv, Wf, bf)
    res = bass_utils.run_bass_kernel_spmd(
        nc,
        in_maps,
        core_ids=list(range(NCORES)),
        trace=_trace,
        trace_cores=_trace_cores,
    )
    out = assemble(res.results)
    kernel.last_result = res
    return out


# revision 18
# speedup vs baseline: 1.2107x; 1.0662x over previous
"""Trainium2 Bass kernel for nn_ExperimentalLayer9 (dense transformer layer).

Layer: x + gelu(attn(x)) @ Wf with
  Q = split_heads(x), K = split_heads(x@Wk+bk), V = split_heads(x@Wv+bv)
  causal softmax (no 1/sqrt(d) scale), exact-erf gelu, residual add.

Sharding over 8 NeuronCores: 2 batch groups x 4-way head/tensor parallel.
Core c handles batch b=c//4 and heads [4r, 4r+4) with r=c%4.

v2 schedule (vs the earlier baseline):
 - q-block-major ("j-major") attention: scores/exp/AV run per (head,
   512-wide q block), so the FF partial for a finished q block and its
   ReduceScatter launch while later attention blocks still compute.
   The RS tail shrinks from ~130us to the last 256-row chunk.
 - FF matmuls run in fp8 (e4m3, DoubleRow perf mode, 2x throughput):
   gelu output and Wf are cast to fp8 (all values are << e4m3 max, no
   scaling needed; measured rel-err ~1.3e-2 < 2e-2 gate).
 - RS in 8 chunks of 256 rows; residual add per chunk right after.
 - SBUF slot sharing: xT halves are reused for got8/xres, wk for wf8.

All other matmuls run in bf16 (fp32 PSUM accumulation); softmax in
fp32/bf16.  exp without max-subtraction (scores are bounded); the
exp-sum l(q) comes free from a ones-column appended to V.
"""

import numpy as np
import ml_dtypes

import concourse.bass as bass
import concourse.mybir as mybir
import concourse.tile as tile
from concourse import bacc
from concourse import bass_utils

# Problem shapes (hardcoded per contest contract).
B, S, D, H, DHID = 2, 2048, 1024, 16, 4096
NCORES = 8
GROUP = 4              # cores per batch group
HPC = 4                # heads per core
DK = 64                # q/k head dim
DV = 256               # v head dim
DKS = HPC * DK         # 256  k-slice per core
DVS = HPC * DV         # 1024 v/hidden slice per core
NM = D // 128          # 8    contraction chunks over d_model
NST = S // 128         # 16   s chunks of 128
VSTRIDE = DV + 1       # 257  V columns per head incl. ones column
NG = 8                 # RS chunks (256 rows each)
GR = S // NG           # 256  rows per RS chunk
ORC = GR // GROUP      # 64   output rows per core per chunk
ROWS = S // GROUP      # 512  output rows per core total

BF16 = mybir.dt.bfloat16
F32 = mybir.dt.float32
FP8 = mybir.dt.float8e4
AF = mybir.ActivationFunctionType
DR = mybir.MatmulPerfMode.DoubleRow

bf16 = ml_dtypes.bfloat16
f8 = ml_dtypes.float8_e4m3

_compiled = None


def build_program():
    nc = bacc.Bacc(
        "TRN2",
        target_bir_lowering=False,
        debug=False,
        enable_asserts=True,
        num_devices=NCORES,
    )

    # Per-core inputs (values differ per core; program is SPMD-identical).
    xT = nc.dram_tensor("xT", [D, S], BF16, kind="ExternalInput").ap()
    qT = nc.dram_tensor("qT", [DKS, S], BF16, kind="ExternalInput").ap()
    xres = nc.dram_tensor("xres", [ROWS, D], F32, kind="ExternalInput").ap()
    wk = nc.dram_tensor("wk", [D, DKS], BF16, kind="ExternalInput").ap()
    wv = nc.dram_tensor("wv", [D, DVS], BF16, kind="ExternalInput").ap()
    wf8 = nc.dram_tensor("wf8", [DVS, D], FP8, kind="ExternalInput").ap()
    bkb = nc.dram_tensor("bkb", [1, DKS], BF16, kind="ExternalInput").ap()
    bvb = nc.dram_tensor("bvb", [1, DVS], BF16, kind="ExternalInput").ap()
    maskt = nc.dram_tensor("maskt", [128, 128], BF16, kind="ExternalInput").ap()
    ident = nc.dram_tensor("ident", [128, 128], BF16, kind="ExternalInput").ap()
    onesr = nc.dram_tensor("onesr", [1, 512], BF16, kind="ExternalInput").ap()
    out = nc.dram_tensor("out", [ROWS, D], F32, kind="ExternalOutput").ap()

    with tile.TileContext(nc) as tc:
        _body(nc, tc, xT, qT, xres, wk, wv, wf8, bkb, bvb, maskt, ident,
              onesr, out)

    nc.compile()
    return nc


def _body(nc, tc, xT, qT, xres, wk, wv, wf8, bkb, bvb, maskt, ident,
          onesr, out):
    with (
        tc.tile_pool(name="const", bufs=1) as constp,
        tc.tile_pool(name="kv", bufs=1) as kvp,
        tc.tile_pool(name="sh1", bufs=1) as sh1p,   # xT_lo <-> got8
        tc.tile_pool(name="sh2", bufs=1) as sh2p,   # xT_hi <-> xres
        tc.tile_pool(name="sh3", bufs=1) as sh3p,   # wk <-> wf8
        tc.tile_pool(name="wvp", bufs=1) as wvp,
        tc.tile_pool(name="expp", bufs=2) as expp,
        tc.tile_pool(name="small", bufs=4) as smallp,
        tc.tile_pool(name="ffout", bufs=4) as ffoutp,
        tc.tile_pool(name="rfp", bufs=2) as rfp,
        tc.tile_pool(name="psS", bufs=2, space="PSUM") as psS,
        tc.tile_pool(name="psV", bufs=2, space="PSUM") as psV,
        tc.tile_pool(name="psG", bufs=2, space="PSUM") as psG,
        tc.tile_pool(name="psT", bufs=2, space="PSUM") as psTp,
        tc.tile_pool(name="dram", bufs=1, space="DRAM") as dramp,
    ):
        # ---- constants (ACT queue) ------------------------------------
        ones_sb = constp.tile([1, 512], BF16)
        nc.scalar.dma_start(ones_sb[:], onesr[:])
        mask_sb = constp.tile([128, 128], BF16)
        nc.scalar.dma_start(mask_sb[:], maskt[:])
        ident_sb = constp.tile([128, 128], BF16)
        nc.scalar.dma_start(ident_sb[:], ident[:])
        bk_sb = constp.tile([1, DKS], BF16)
        nc.scalar.dma_start(bk_sb[:], bkb[:])
        bv_sb = constp.tile([1, DVS], BF16)
        nc.scalar.dma_start(bv_sb[:], bvb[:])

        # Warm up the collectives path (ncfw/channel setup) so the first
        # real ReduceScatter doesn't pay ~25us of first-call overhead.
        warm_in = dramp.tile([4, 16], BF16, tag="warm_in")
        warm_out = dramp.tile([1, 16], BF16, tag="warm_out")
        nc.scalar.dma_start(
            warm_in[:].rearrange("a b -> (a b)")[None, :], ones_sb[0:1, 0:64]
        )
        nc.gpsimd.collective_compute(
            "ReduceScatter",
            mybir.AluOpType.add,
            replica_groups=[[0, 1, 2, 3], [4, 5, 6, 7]],
            ins=[warm_in.opt()],
            outs=[warm_out.opt()],
        )

        # ---- input loads (Sync queue; K-proj deps first) --------------
        wk_sb = sh3p.tile([128, NM, DKS], BF16, tag="sh3t")
        for m in range(NM):
            nc.sync.dma_start(wk_sb[:, m, :], wk[m * 128 : (m + 1) * 128, :])
        xT_lo = sh1p.tile([128, NM, 1024], BF16, tag="sh1t")
        xT_hi = sh2p.tile([128, NM, 1024], BF16, tag="sh2t")
        for m in range(NM):
            nc.sync.dma_start(xT_lo[:, m, :], xT[m * 128 : (m + 1) * 128, 0:1024])
        qT_sb = kvp.tile([128, 2, S], BF16)
        for dkt in range(2):
            nc.sync.dma_start(qT_sb[:, dkt, :], qT[dkt * 128 : (dkt + 1) * 128, :])
        for m in range(NM):
            nc.sync.dma_start(xT_hi[:, m, :], xT[m * 128 : (m + 1) * 128, 1024:2048])
        wv_sb = wvp.tile([128, NM, DVS], BF16)
        for m in range(NM):
            nc.sync.dma_start(wv_sb[:, m, :], wv[m * 128 : (m + 1) * 128, :])

        kt_sb = kvp.tile([128, 2, S], BF16)   # K^T rows dk%128, chunk dk//128
        v_sb = kvp.tile([128, NST, HPC, VSTRIDE], BF16)

        def xT_chunk(m, st):   # [128, 128] chunk of x^T: rows m, cols s-tile st
            t, off = (xT_lo, st) if st < 8 else (xT_hi, st - 8)
            return t[:, m, off * 128 : (off + 1) * 128]

        def xT_s512(m, sg):    # [128, 512] chunk: rows m, s-512-block sg
            t, off = (xT_lo, sg) if sg < 2 else (xT_hi, sg - 2)
            return t[:, m, off * 512 : (off + 1) * 512]

        # ---- K^T projection (st-major so scores can start early) ------
        for sg in range(4):
            for dkt in range(2):
                ps = psG.tile([128, 512], F32, tag="gen")
                nc.tensor.matmul(
                    ps[:], bk_sb[:, dkt * 128 : (dkt + 1) * 128],
                    ones_sb[:, 0:512], start=True, stop=False,
                )
                for m in range(NM):
                    nc.tensor.matmul(
                        ps[:],
                        wk_sb[:, m, dkt * 128 : dkt * 128 + 128],
                        xT_s512(m, sg),
                        start=False, stop=(m == NM - 1),
                    )
                nc.scalar.copy(kt_sb[:, dkt, sg * 512 : (sg + 1) * 512], ps[:])

        # ---- V projection ---------------------------------------------
        # v_sb[s, st, h, 0:256] = V values, col 256 = 1.0 (for the exp-sum)
        nc.vector.memset(v_sb[:, :, :, DV], 1.0)

        def v_proj(st_range):
            for st in st_range:
                for dvh in range(2):  # dv halves of 512 = heads (2dvh, 2dvh+1)
                    ps = psG.tile([128, 512], F32, tag="gen")
                    nc.tensor.matmul(
                        ps[:], ones_sb[:, 0:128],
                        bv_sb[:, dvh * 512 : dvh * 512 + 512],
                        start=True, stop=False,
                    )
                    for m in range(NM):
                        nc.tensor.matmul(
                            ps[:], xT_chunk(m, st),
                            wv_sb[:, m, dvh * 512 : dvh * 512 + 512],
                            start=False, stop=(m == NM - 1),
                        )
                    for hh in range(2):
                        nc.scalar.copy(
                            v_sb[:, st, 2 * dvh + hh, 0:DV],
                            ps[:, hh * 256 : hh * 256 + 256],
                        )

        # ---- attention pieces ------------------------------------------
        # scores^T[k, q] per (head, 512-q-block qg, 128-k-chunk kt), exp'd
        # into exps[:, kt, :]; causal diagonal masked by mask_sb multiply.
        def scores_block(h, qg, exps):
            po = 64 * (h % 2)
            dkt = h // 2
            q0 = qg * 512
            for kt in range(4 * qg + 4):
                t = kt - 4 * qg
                toff = max(t, 0) * 128
                ps = psS.tile([128, 512], F32, tag="st")
                nc.tensor.matmul(
                    ps[:, toff:512],
                    kt_sb[po : po + 64, dkt, kt * 128 : kt * 128 + 128],
                    qT_sb[po : po + 64, dkt, q0 + toff : q0 + 512],
                    start=True, stop=True,
                    tile_position=(po, 0),
                )
                nc.scalar.activation(
                    exps[:, kt, toff:512], ps[:, toff:512], AF.Exp
                )
                if t >= 0:
                    blk = exps[:, kt, toff : toff + 128]
                    nc.vector.tensor_mul(blk, blk, mask_sb[:])

        # deferred o^T tiles: PE-transpose (identity matmul) + gelu from
        # PSUM straight to fp8.  Emitted one AV chain late so the tensor
        # queue never waits on the DVE normalize of the current tile.
        pending = []

        def flush_pending():
            while pending:
                h, i, ot = pending.pop(0)
                for half in range(2):
                    psT = psTp.tile([128, 128], BF16, tag="pt")
                    nc.tensor.transpose(
                        psT[:], ot[:, half * 128 : half * 128 + 128], ident_sb[:]
                    )
                    nc.scalar.activation(
                        got8[:, 2 * h + half, i * 128 : i * 128 + 128],
                        psT[:], AF.Gelu,
                    )

        def av_block(h, qg, exps):
            for sq in range(4):
                i = 4 * qg + sq
                pso = psV.tile([128, VSTRIDE], F32, tag="av")
                for kt in range(i + 1):
                    nc.tensor.matmul(
                        pso[:],
                        exps[:, kt, sq * 128 : sq * 128 + 128],
                        v_sb[:, kt, h, :],
                        start=(kt == 0), stop=(kt == i),
                    )
                flush_pending()
                recip = smallp.tile([128, 1], F32, tag="recip")
                nc.vector.reciprocal(recip[:], pso[:, DV : DV + 1])
                ot = smallp.tile([128, DV], BF16, tag="ot")
                nc.vector.tensor_scalar_mul(ot[:], pso[:, 0:DV], recip[:])
                pending.append((h, i, ot))

        # ---- FF chunk (256 q rows) + RS + residual ---------------------
        def ff_chunk(g, got8, xres_sb):
            partial_d = dramp.tile([GR, D], BF16, tag=f"part{g}")
            for cc in range(2):
                c = 2 * g + cc
                ps0 = psG.tile([128, 512], F32, tag="gen")
                ps1 = psG.tile([128, 512], F32, tag="gen")
                for dc in range(4):
                    lhsT = got8[:, 2 * dc : 2 * dc + 2, c * 128 : c * 128 + 128]
                    nc.tensor.matmul(
                        ps0[:], lhsT, wf8_sb[:, 2 * dc : 2 * dc + 2, 0:512],
                        start=(dc == 0), stop=(dc == 3), perf_mode=DR,
                    )
                    nc.tensor.matmul(
                        ps1[:], lhsT, wf8_sb[:, 2 * dc : 2 * dc + 2, 512:1024],
                        start=(dc == 0), stop=(dc == 3), perf_mode=DR,
                    )
                fo = ffoutp.tile([128, D], BF16, tag="ffout")
                nc.vector.tensor_copy(fo[:, 0:512], ps0[:])
                nc.vector.tensor_copy(fo[:, 512:1024], ps1[:])
                nc.scalar.dma_start(partial_d[cc * 128 : (cc + 1) * 128, :], fo[:])
            rs_d = dramp.tile([ORC, D], BF16, tag=f"rs{g}")
            nc.gpsimd.collective_compute(
                "ReduceScatter",
                mybir.AluOpType.add,
                replica_groups=[[0, 1, 2, 3], [4, 5, 6, 7]],
                ins=[partial_d.opt()],
                outs=[rs_d.opt()],
            )
            return rs_d

        def residual(g, rs_d, xres_sb):
            # RS-gated cast-DMA on the GpSimd queue, add on DVE, store
            # on ACT.  Deferred to the end of the program so the blocking
            # wait on the RS never stalls mid-kernel DVE/ACT work.
            po = 64 * (g % 2)
            rf = rfp.tile([128, D], F32, tag="rf")
            nc.gpsimd.dma_start(rf[po : po + ORC, :], rs_d[:])
            xr = xres_sb[po : po + ORC, g // 2, :]
            nc.vector.tensor_add(xr, xr, rf[po : po + ORC, :])
            nc.scalar.dma_start(out[g * ORC : (g + 1) * ORC, :], xr)

        def new_exps():
            exps = expp.tile([128, 16, 512], BF16, tag="exps")
            return exps

        # ================= schedule =====================================
        # Per 512-q-block qg: attention for all heads, then gelu+FF+RS for
        # its two 256-row chunks while the next block's attention runs.
        # Tensor-queue filler (next block's h0 scores / late V projection)
        # covers the o^T-transpose latency before each FF.
        v_proj(range(8))          # V for k < 1024 (feeds qg 0,1)

        # wk dead after K proj: load wf8 into the shared slot (gpsimd q)
        wf8_sb = sh3p.tile([128, NM, D], FP8, tag="sh3t")
        for m in range(NM):
            nc.gpsimd.dma_start(wf8_sb[:, m, :], wf8[m * 128 : (m + 1) * 128, :])
        got8 = sh1p.tile([128, NM, S], FP8, tag="sh1t")
        xres_sb = sh2p.tile([128, 4, D], F32, tag="sh2t")

        rs_ds = [None] * NG

        def do_ff(qg):
            for g in (2 * qg, 2 * qg + 1):
                rs_ds[g] = ff_chunk(g, got8, xres_sb)

        # ---- qg0 ----
        e0 = new_exps()
        scores_block(0, 0, e0)
        av_block(0, 0, e0)
        for h in range(1, HPC):
            e = new_exps()
            scores_block(h, 0, e)
            av_block(h, 0, e)
        e0 = new_exps()
        scores_block(0, 1, e0)     # prefetch next block's h0
        v_proj(range(8, 16))       # V for k >= 1024
        # xT dead now: xres into the shared slot (gpsimd queue)
        for g4 in range(4):
            nc.gpsimd.dma_start(
                xres_sb[:, g4, :], xres[g4 * 128 : (g4 + 1) * 128, :]
            )
        flush_pending()
        do_ff(0)

        # ---- qg1..qg3: FF of block qg-1 is issued mid-attention of qg,
        # where its gelu/transposes are long since complete.
        for qg in range(1, 4):
            av_block(0, qg, e0)
            for h in range(1, HPC):
                e = new_exps()
                scores_block(h, qg, e)
                if h == 1 and qg >= 2:
                    do_ff(qg - 1)
                if h == 3 and qg < 3:
                    e0 = new_exps()
                    scores_block(0, qg + 1, e0)
                av_block(h, qg, e)
        flush_pending()
        do_ff(3)

        # ---- residuals (all RS waits concentrated at the end) ----------
        for g in range(NG):
            residual(g, rs_ds[g], xres_sb)


def make_in_maps(x, Wk, bk, Wv, bv, Wf, bf):
    """Host-side sharding: returns the per-core input dict list."""
    x = np.asarray(x, np.float32)
    Wk = np.asarray(Wk, np.float32)
    Wv = np.asarray(Wv, np.float32)
    Wf = np.asarray(Wf, np.float32)
    bk = np.asarray(bk, np.float32)
    bv = np.asarray(bv, np.float32)
    bf = np.asarray(bf, np.float32)
    mask = np.tril(np.ones((128, 128), np.float32)).T  # mask[k,q]=1 iff k<=q
    in_maps = []
    for c in range(NCORES):
        b, r = c // GROUP, c % GROUP
        xb = x[b]                                    # [S, D]
        xT = np.ascontiguousarray(xb.T).astype(bf16)
        qTs = xT[DKS * r : DKS * (r + 1)]            # heads 4r..4r+3 rows
        # chunked RS: core (b,r) chunk g holds x rows 256g+64r+[0,64)
        xres = np.concatenate(
            [xb[GR * g + ORC * r : GR * g + ORC * r + ORC] for g in range(NG)]
        ) + bf[None, :].astype(np.float32)
        in_maps.append({
            "xT": xT,
            "qT": np.ascontiguousarray(qTs),
            "xres": np.ascontiguousarray(xres),
            "wk": np.ascontiguousarray(Wk[:, DKS * r : DKS * (r + 1)]).astype(bf16),
            "wv": np.ascontiguousarray(Wv[:, DVS * r : DVS * (r + 1)]).astype(bf16),
            "wf8": np.ascontiguousarray(Wf[DVS * r : DVS * (r + 1), :]).astype(f8),
            "bkb": bk[None, DKS * r : DKS * (r + 1)].astype(bf16),
            "bvb": bv[None, DVS * r : DVS * (r + 1)].astype(bf16),
            "maskt": mask.astype(bf16),
            "ident": np.eye(128, dtype=np.float32).astype(bf16),
            "onesr": np.ones((1, 512), bf16),
        })
    return in_maps


def assemble(results):
    """[8 x [512,1024]] core outputs -> [2,2048,1024]."""
    out = np.empty((B, S, D), np.float32)
    for c in range(NCORES):
        b, r = c // GROUP, c % GROUP
        for g in range(NG):
            out[b, GR * g + ORC * r : GR * g + ORC * r + ORC, :] = results[c][
                "out"
            ][ORC * g : ORC * (g + 1)]
    return out


def kernel(x, Wk, bk, Wv, bv, Wf, bf, _trace=False, _trace_cores=None):
    global _compiled
    if _compiled is None:
        _compiled = build_program()
    nc = _compiled
    in_maps = make_in_maps(x, Wk, bk, Wv, bv, Wf, bf)
    res = bass_utils.run_bass_kernel_spmd(
        nc,
        in_maps,
        core_ids=list(range(NCORES)),
        trace=_trace,
        trace_cores=_trace_cores,
    )
    out = assemble(res.results)
    kernel.last_result = res
    return out
